# revision 5
# baseline (speedup 1.0000x reference)
"""Bass/Tile program builder for nn_GatedDeltaRecurrence on 8 trn2 cores.

Per-core sharding: b = core//4, heads = (2*(core%4), 2*(core%4)+1).
Phases: A projections+conv+norm, B chunked gated-delta scan (C=128,
triangular inverse via nilpotent power doubling), C AllReduce RMS, D out GEMM.
All matmul operands bf16; PSUM accumulation and master state f32.
Host sums the 4 partial outputs per b.
"""
import numpy as np

import concourse.bass as bass
import concourse.mybir as mybir
from concourse import bacc
from concourse.tile import TileContext
from concourse.masks import make_identity

F32 = mybir.dt.float32
BF16 = mybir.dt.bfloat16
AF = mybir.ActivationFunctionType
ALU = mybir.AluOpType

T = 2048
C = 128
NCH = T // C
KH = 96
VH = 192
DX = 1024
DKV = 512
EPS = 1e-6
TB = 512


def build_program():
    nc = bacc.Bacc()

    xT = nc.declare_dram_parameter("xT", [DX, T], BF16, isOutput=False)
    ckvT = nc.declare_dram_parameter("ckvT", [DKV, T], BF16, isOutput=False)
    wq_d = nc.declare_dram_parameter("wq", [DX, 192], BF16, isOutput=False)
    wk_d = nc.declare_dram_parameter("wk", [DKV, 192], BF16, isOutput=False)
    wvs_d = nc.declare_dram_parameter("wvs", [4, DKV, 384], BF16, isOutput=False)
    wg_d = nc.declare_dram_parameter("wg", [DX, 384], BF16, isOutput=False)
    wo_d = nc.declare_dram_parameter("wo", [384, DX], BF16, isOutput=False)
    wab_d = nc.declare_dram_parameter("wab", [DX, 4], BF16, isOutput=False)
    convq_d = nc.declare_dram_parameter("convq", [192, 4], F32, isOutput=False)
    convk_d = nc.declare_dram_parameter("convk", [192, 4], F32, isOutput=False)
    cbq_d = nc.declare_dram_parameter("cbq", [192], F32, isOutput=False)
    cbk_d = nc.declare_dram_parameter("cbk", [192], F32, isOutput=False)
    vbias_d = nc.declare_dram_parameter("vbias", [384], F32, isOutput=False)
    # abc: col0 = sigmoid bias rows2:4 (b_proj_b);
    # col1 = -(dt_bias + a_proj_b) rows0:2 (for -softplus(x) = ln(sigmoid(-x)));
    # col2 = +exp(A_log) rows0:2
    abc_d = nc.declare_dram_parameter("abc", [2, 3], F32, isOutput=False)
    pw_d = nc.declare_dram_parameter("pw", [384], F32, isOutput=False)
    masks_d = nc.declare_dram_parameter("masks", [2, C, C], F32, isOutput=False)
    out_d = nc.declare_dram_parameter("out", [T, DX], F32, isOutput=True)
    DBG = bool(__import__("os").environ.get("KB_DEBUG"))
    if DBG:
        dbg_qn = nc.declare_dram_parameter("dbg_qn", [96, 2 * T], BF16, isOutput=True)
        dbg_kn = nc.declare_dram_parameter("dbg_kn", [96, 2 * T], BF16, isOutput=True)
        dbg_v = nc.declare_dram_parameter("dbg_v", [128, NCH * 384], BF16, isOutput=True)
        dbg_gba = nc.declare_dram_parameter("dbg_gba", [2, 2 * T], F32, isOutput=True)
        dbg_tab = nc.declare_dram_parameter("dbg_tab", [128, 128], F32, isOutput=True)
        dbg_rows = nc.declare_dram_parameter("dbg_rows", [1, 2 * 4096], F32, isOutput=True)
        dbg_oT = nc.declare_dram_parameter("dbg_oT", [96, 4 * T], F32, isOutput=True)
        dbg_ss = nc.declare_dram_parameter("dbg_ss", [1, 3 * T], F32, isOutput=True)
        dbg_gate = nc.declare_dram_parameter("dbg_gate", [96, 4 * T], BF16, isOutput=True)

    gb_scr = nc.dram_tensor("gb_scr", [4, T], F32)
    cc_in = nc.dram_tensor("cc_in", [1, T], F32)
    cc_out = nc.dram_tensor("cc_out", [1, T], F32)
    RG = [[0, 1, 2, 3], [4, 5, 6, 7]]

    with TileContext(nc) as tc:
        with (
            tc.tile_pool(name="per", bufs=1) as per,
            tc.tile_pool(name="ps", bufs=4, space="PSUM") as ps,
            tc.tile_pool(name="ps1", bufs=2, space="PSUM") as ps1,
            tc.tile_pool(name="pso", bufs=2, space="PSUM") as pso,
        ):
            # ---------------- small persistents ----------------
            convq_sb = per.tile([96, 2, 4], F32, tag="convq_sb")
            nc.sync.dma_start(out=convq_sb,
                              in_=convq_d.rearrange("(h p) s -> p h s", p=96))
            convk_sb = per.tile([96, 2, 4], F32, tag="convk_sb")
            nc.sync.dma_start(out=convk_sb,
                              in_=convk_d.rearrange("(h p) s -> p h s", p=96))
            cbq_sb = per.tile([96, 2], F32, tag="cbq_sb")
            nc.sync.dma_start(out=cbq_sb, in_=cbq_d.rearrange("(h p) -> p h", p=96))
            cbk_sb = per.tile([96, 2], F32, tag="cbk_sb")
            nc.sync.dma_start(out=cbk_sb, in_=cbk_d.rearrange("(h p) -> p h", p=96))
            abc_sb = per.tile([2, 3], F32, tag="abc_sb")
            nc.sync.dma_start(out=abc_sb, in_=abc_d[:])
            pw_sb = per.tile([96, 4], F32, tag="pw_sb")
            nc.sync.dma_start(out=pw_sb, in_=pw_d.rearrange("(q p) -> p q", p=96))
            mst_sb = per.tile([C, C], F32, tag="mst_sb")
            nc.sync.dma_start(out=mst_sb, in_=masks_d[0])
            mui_sb = per.tile([C, C], F32, tag="mui_sb")
            nc.sync.dma_start(out=mui_sb, in_=masks_d[1])
            vb_row = per.tile([1, 384], F32, tag="vb_row")
            nc.sync.dma_start(out=vb_row, in_=vbias_d[None, :])
            vb_bc = per.tile([128, 384], F32, tag="vb_bc")
            nc.gpsimd.partition_broadcast(vb_bc, vb_row)

            ones96 = per.tile([96, 1], BF16, tag="ones96")
            nc.vector.memset(ones96, 1.0)
            id_bf = per.tile([128, 128], BF16, tag="id_bf")
            make_identity(nc, id_bf)
            id_f32 = per.tile([128, 128], F32, tag="id_f32")
            make_identity(nc, id_f32)

            # persistent activations / tables
            qn = [per.tile([96, T], BF16, tag=f"qn{h}", name=f"qn{h}")
                  for h in range(2)]
            kn = [per.tile([96, T], BF16, tag=f"kn{h}", name=f"kn{h}")
                  for h in range(2)]
            v_sb = per.tile([128, NCH, 384], BF16, tag="v_sb")
            gate_sb = per.tile([96, 4, T], BF16, tag="gate_sb")
            oT = per.tile([96, 4, T], F32, tag="oT")
            wo_sb = per.tile([96, 4, DX], BF16, tag="wo_sb")
            nc.sync.dma_start(out=wo_sb, in_=wo_d.rearrange("(q p) d -> p q d", p=96))

            cg_all = per.tile([128, 16, 2], F32, tag="cg_all")
            beta_all = per.tile([128, 16, 2], F32, tag="beta_all")
            b_all = per.tile([128, 16, 2], F32, tag="b_all")
            r_all = per.tile([128, 16, 2], F32, tag="r_all")

            S0 = per.tile([96, 2, VH], F32, tag="S0")
            nc.vector.memset(S0, 0.0)
            S0_bf = per.tile([96, 2, VH], BF16, tag="S0_bf")
            nc.vector.memset(S0_bf, 0.0)

            # ================= phase A =================
            with (
                tc.tile_pool(name="saw", bufs=1) as saw,
                tc.tile_pool(name="sat", bufs=2) as sat,
                tc.tile_pool(name="sa1", bufs=1) as sa1,
            ):
                wq_sb = saw.tile([128, 8, 192], BF16, tag="wq_sb")
                nc.sync.dma_start(out=wq_sb,
                                  in_=wq_d.rearrange("(c p) m -> p c m", p=128))
                wk_sb = saw.tile([128, 4, 192], BF16, tag="wk_sb")
                nc.sync.dma_start(out=wk_sb,
                                  in_=wk_d.rearrange("(c p) m -> p c m", p=128))
                wvs_sb = saw.tile([128, 4, 4, 384], BF16, tag="wvs_sb")
                nc.sync.dma_start(out=wvs_sb,
                                  in_=wvs_d.rearrange("s (c p) m -> p s c m", p=128))
                wg_sb = saw.tile([128, 8, 384], BF16, tag="wg_sb")
                nc.sync.dma_start(out=wg_sb,
                                  in_=wg_d.rearrange("(c p) m -> p c m", p=128))
                wab_sb = saw.tile([128, 8, 4], BF16, tag="wab_sb")
                nc.sync.dma_start(out=wab_sb,
                                  in_=wab_d.rearrange("(c p) m -> p c m", p=128))
                gba = sa1.tile([2, 2, T], F32, tag="gba")
                rawq = [sat.tile([96, T], BF16, tag="raw", name=f"rawq{h}")
                        for h in range(2)]

                # ---- q + ab pass over xT slabs ----
                for tb in range(4):
                    tsl = slice(tb * TB, (tb + 1) * TB)
                    xslab = sat.tile([128, 8, TB], BF16, tag="xslab")
                    nc.sync.dma_start(
                        out=xslab,
                        in_=xT.rearrange("(c p) t -> p c t", p=128)[:, :, tsl])
                    pq = [ps.tile([96, TB], F32, tag="mm", name=f"pq{h}")
                          for h in range(2)]
                    pa = ps.tile([2, TB], F32, tag="mm", name="pa")
                    pb = ps.tile([2, TB], F32, tag="mm", name="pb")
                    for d in range(8):
                        for h in range(2):
                            nc.tensor.matmul(pq[h],
                                             lhsT=wq_sb[:, d, h*96:(h+1)*96],
                                             rhs=xslab[:, d, :],
                                             start=(d == 0), stop=(d == 7))
                        nc.tensor.matmul(pa, lhsT=wab_sb[:, d, 0:2],
                                         rhs=xslab[:, d, :],
                                         start=(d == 0), stop=(d == 7))
                        nc.tensor.matmul(pb, lhsT=wab_sb[:, d, 2:4],
                                         rhs=xslab[:, d, :],
                                         start=(d == 0), stop=(d == 7))
                    for h in range(2):
                        nc.vector.tensor_copy(out=rawq[h][:, tsl], in_=pq[h])
                    nc.scalar.activation(gba[:, 0, tsl], pa, AF.Sigmoid,
                                         bias=abc_sb[:, 0:1], scale=-1.0)
                    nc.scalar.activation(gba[:, 0, tsl], gba[:, 0, tsl], AF.Ln)
                    nc.vector.tensor_scalar(out=gba[:, 0, tsl], in0=gba[:, 0, tsl],
                                            scalar1=abc_sb[:, 1:2], scalar2=None,
                                            op0=ALU.mult)
                    nc.scalar.activation(gba[:, 1, tsl], pb, AF.Sigmoid,
                                         bias=abc_sb[:, 2:3], scale=1.0)

                def conv_norm(raws, cw, cb, dsts, extra):
                    for h in range(2):
                        raw = raws[h]
                        dst = dsts[h]
                        nc.vector.tensor_scalar(out=dst, in0=raw,
                                                scalar1=cw[:, h, 3:4],
                                                scalar2=None, op0=ALU.mult)
                        for s in (2, 1, 0):
                            sh = 3 - s
                            nc.vector.scalar_tensor_tensor(
                                out=dst[:, sh:], in0=raw[:, :T - sh],
                                scalar=cw[:, h, s:s + 1], in1=dst[:, sh:],
                                op0=ALU.mult, op1=ALU.add)
                        nc.scalar.activation(dst, dst, AF.Silu,
                                             bias=cb[:, h:h + 1], scale=1.0)
                        sq = sa1.tile([96, T], BF16, tag="sq")
                        nc.vector.tensor_tensor(sq, dst, dst, ALU.mult)
                        rn = sa1.tile([1, T], F32, tag="rn")
                        for tb in range(4):
                            tsl = slice(tb * TB, (tb + 1) * TB)
                            pnrm = ps1.tile([1, TB], F32, tag="aux")
                            nc.tensor.matmul(pnrm, lhsT=ones96, rhs=sq[:, tsl],
                                             start=True, stop=True)
                            nc.scalar.activation(rn[:, tsl], pnrm, AF.Sqrt)
                        nc.vector.tensor_scalar(out=rn, in0=rn,
                                                scalar1=float(EPS), scalar2=None,
                                                op0=ALU.add)
                        nc.vector.reciprocal(rn, rn)
                        if extra != 1.0:
                            nc.vector.tensor_scalar(out=rn, in0=rn, scalar1=extra,
                                                    scalar2=None, op0=ALU.mult)
                        rn_bc = sa1.tile([96, T], F32, tag="rn_bc")
                        nc.gpsimd.partition_broadcast(rn_bc, rn)
                        nc.vector.tensor_tensor(dst, dst, rn_bc, ALU.mult)

                conv_norm(rawq, convq_sb, cbq_sb, qn, float(KH) ** -0.5)

                # ---- k + v pass over ckv slabs (padded by 3 zero cols) ----
                rawk = [sat.tile([96, T], BF16, tag="raw", name=f"rawk{h}")
                        for h in range(2)]
                for tb in range(4):
                    tsl = slice(tb * TB, (tb + 1) * TB)
                    cslab = sat.tile([128, 4, TB + 3], BF16, tag="cslab")
                    src = ckvT.rearrange("(c p) t -> p c t", p=128)
                    if tb == 0:
                        nc.vector.memset(cslab[:, :, 0:3], 0.0)
                        nc.sync.dma_start(out=cslab[:, :, 3:], in_=src[:, :, 0:TB])
                    else:
                        nc.sync.dma_start(out=cslab,
                                          in_=src[:, :, tb*TB-3:(tb+1)*TB])
                    pk = [ps.tile([96, TB], F32, tag="mm", name=f"pk{h}")
                          for h in range(2)]
                    for d in range(4):
                        for h in range(2):
                            nc.tensor.matmul(pk[h],
                                             lhsT=wk_sb[:, d, h*96:(h+1)*96],
                                             rhs=cslab[:, d, 3:],
                                             start=(d == 0), stop=(d == 3))
                    for h in range(2):
                        nc.vector.tensor_copy(out=rawk[h][:, tsl], in_=pk[h])
                    # v: 4 chunk-tiles per slab, 4-shift conv in PSUM
                    for vt_l in range(4):
                        vt = tb * 4 + vt_l
                        pv = ps.tile([128, 384], F32, tag="mm")
                        first = True
                        for s in (3, 2, 1, 0):
                            for d in range(4):
                                lo = vt_l * 128 + s
                                nc.tensor.matmul(pv,
                                                 lhsT=cslab[:, d, lo:lo + 128],
                                                 rhs=wvs_sb[:, s, d, :],
                                                 start=first,
                                                 stop=(s == 0 and d == 3))
                                first = False
                        vtmp = sat.tile([128, 384], F32, tag="vtmp")
                        nc.vector.tensor_tensor(vtmp, pv, vb_bc, ALU.add)
                        nc.scalar.activation(v_sb[:, vt, :], vtmp, AF.Silu)


                conv_norm(rawk, convk_sb, cbk_sb, kn, 1.0)

                # ---- gate pass over xT slabs ----
                for tb in range(4):
                    tsl = slice(tb * TB, (tb + 1) * TB)
                    xslab = sat.tile([128, 8, TB], BF16, tag="xslab")
                    nc.sync.dma_start(
                        out=xslab,
                        in_=xT.rearrange("(c p) t -> p c t", p=128)[:, :, tsl])
                    for q in range(4):
                        pg = ps.tile([96, TB], F32, tag="mm")
                        for d in range(8):
                            nc.tensor.matmul(pg, lhsT=wg_sb[:, d, q*96:(q+1)*96],
                                             rhs=xslab[:, d, :],
                                             start=(d == 0), stop=(d == 7))
                        nc.scalar.activation(gate_sb[:, q, tsl], pg, AF.Silu)

                if DBG:
                    for h in range(2):
                        nc.sync.dma_start(out=dbg_qn.rearrange("p (h t) -> p h t", h=2)[:, h, :], in_=qn[h])
                        nc.sync.dma_start(out=dbg_kn.rearrange("p (h t) -> p h t", h=2)[:, h, :], in_=kn[h])
                    nc.sync.dma_start(out=dbg_v.rearrange("p (c m) -> p c m", c=NCH), in_=v_sb)
                    nc.sync.dma_start(out=dbg_gba.rearrange("p (k t) -> p k t", k=2), in_=gba)
                    nc.sync.dma_start(out=dbg_gate.rearrange("p (q t) -> p q t", q=4), in_=gate_sb)

                # ---- g/beta chunk tables ----
                cg_rows = sa1.tile([2, T], F32, tag="cg_rows")
                nc.vector.tensor_tensor_scan(cg_rows, gba[:, 0, :], gba[:, 0, :],
                                             0.0, op0=ALU.add, op1=ALU.bypass)
                nc.sync.dma_start(out=gb_scr[0:2, :], in_=cg_rows)
                nc.sync.dma_start(out=gb_scr[2:4, :], in_=gba[:, 1, :])

            # ================= phase B: chunked scan =================
            with (
                tc.tile_pool(name="sb1", bufs=1) as sb1,
                tc.tile_pool(name="wkp", bufs=2) as wkp,
                tc.tile_pool(name="lup", bufs=2) as lup,
            ):
                cgT_rows = sb1.tile([1, 2, 16, 128], F32, tag="cgT_rows")
                bT_rows = sb1.tile([1, 2, 16, 128], F32, tag="bT_rows")
                for h in range(2):
                    nc.sync.dma_start(
                        out=cg_all[:, :, h],
                        in_=gb_scr[h].rearrange("(c r) -> r c", r=128))
                    nc.sync.dma_start(
                        out=beta_all[:, :, h],
                        in_=gb_scr[2 + h].rearrange("(c r) -> r c", r=128))
                    nc.sync.dma_start(
                        out=cgT_rows[:, h],
                        in_=gb_scr[h].rearrange("(c r) -> c r", r=128))
                prev_hc = sb1.tile([1, 2, 16], F32, tag="prev_hc")
                nc.vector.memset(prev_hc[:, :, 0:1], 0.0)
                for h in range(2):
                    nc.sync.dma_start(
                        out=prev_hc[:, h, 1:16],
                        in_=gb_scr[h].rearrange("(c r) -> c r", r=128)[0:15,
                                                                      127:128])
                cg_view = cgT_rows.rearrange("p h c r -> p (h c) r")
                nc.vector.tensor_tensor(
                    cg_view, cg_view,
                    prev_hc.rearrange("p h c -> p (h c)")[:, :, None]
                    .to_broadcast((1, 32, 128)), ALU.subtract)
                nc.scalar.activation(bT_rows.rearrange("p h c r -> p (h c) r"),
                                     cg_view, AF.Exp)
                prev_row = sb1.tile([1, 16, 2], F32, tag="prev_row")
                nc.vector.memset(prev_row[:, 0:1, :], 0.0)
                lastrow = sb1.tile([1, 16, 2], F32, tag="lastrow")
                for h in range(2):
                    nc.sync.dma_start(
                        out=prev_row[:, 1:16, h],
                        in_=gb_scr[h].rearrange("(c r) -> c r", r=128)[0:15,
                                                                      127:128])
                    nc.sync.dma_start(
                        out=lastrow[:, :, h],
                        in_=gb_scr[h].rearrange("(c r) -> c r", r=128)[0:16,
                                                                      127:128])
                prev_bc = sb1.tile([128, 16, 2], F32, tag="prev_bc")
                nc.gpsimd.partition_broadcast(prev_bc, prev_row)
                nc.vector.tensor_tensor(cg_all, cg_all, prev_bc, ALU.subtract)
                nc.scalar.activation(b_all, cg_all, AF.Exp)
                rlast_row = sb1.tile([1, 16, 2], F32, tag="rlast_row")
                nc.vector.tensor_tensor(rlast_row, lastrow, prev_row,
                                        ALU.subtract)
                rlast_bc = sb1.tile([128, 16, 2], F32, tag="rlast_bc")
                nc.gpsimd.partition_broadcast(rlast_bc, rlast_row)
                nc.vector.scalar_tensor_tensor(out=r_all, in0=cg_all, scalar=-1.0,
                                               in1=rlast_bc, op0=ALU.mult,
                                               op1=ALU.add)
                nc.scalar.activation(r_all, r_all, AF.Exp)
                if DBG:
                    dv = dbg_tab.rearrange("p (k c h) -> p k c h", k=4, c=16)
                    nc.sync.dma_start(out=dv[:, 0], in_=cg_all)
                    nc.sync.dma_start(out=dv[:, 1], in_=beta_all)
                    nc.sync.dma_start(out=dv[:, 2], in_=b_all)
                    nc.sync.dma_start(out=dv[:, 3], in_=r_all)
                    dr = dbg_rows.rearrange("p (k x) -> p k x", k=2)
                    nc.sync.dma_start(out=dr[:, 0], in_=cgT_rows.rearrange("p h c r -> p (h c r)"))
                    nc.sync.dma_start(out=dr[:, 1], in_=bT_rows.rearrange("p h c r -> p (h c r)"))

                for ci in range(NCH):
                    j0 = 2 * ci
                    csl = slice(ci * C, (ci + 1) * C)
                    cg_pair = cg_all[:, ci, :]
                    beta_pair = beta_all[:, ci, :]
                    b_pair = b_all[:, ci, :]
                    r_pair = r_all[:, ci, :]

                    growp = wkp.tile([128, 2, C], F32, tag="growp")
                    for h in range(2):
                        nc.gpsimd.partition_broadcast(
                            growp[:, h, :], cgT_rows[0:1, h, ci, :])
                    # dd[p,f] = cg_f - cg_p
                    dd = wkp.tile([128, 2, C], F32, tag="dd")
                    nc.vector.tensor_tensor(
                        dd, growp, cg_pair[:, :, None].to_broadcast((128, 2, C)),
                        ALU.subtract)
                    d1 = wkp.tile([128, 2, C], F32, tag="d1")
                    nc.vector.tensor_scalar(out=d1, in0=dd, scalar1=0.0,
                                            scalar2=-1.0, op0=ALU.max,
                                            op1=ALU.mult)
                    e_l = wkp.tile([128, 2, C], F32, tag="e_l")
                    nc.scalar.activation(e_l, d1, AF.Exp)
                    d2 = wkp.tile([128, 2, C], F32, tag="d2")
                    nc.vector.tensor_scalar(out=d2, in0=dd, scalar1=0.0,
                                            scalar2=None, op0=ALU.min)
                    e_t = wkp.tile([128, 2, C], F32, tag="e_t")
                    nc.scalar.activation(e_t, d2, AF.Exp)

                    pkk = ps.tile([128, 2, C], F32, tag="mm")
                    for h in range(2):
                        nc.tensor.matmul(pkk[:, h, :], lhsT=kn[h][:, csl],
                                         rhs=kn[h][:, csl], start=True, stop=True)
                    pqk = ps.tile([128, 2, C], F32, tag="mm")
                    for h in range(2):
                        nc.tensor.matmul(pqk[:, h, :], lhsT=kn[h][:, csl],
                                         rhs=qn[h][:, csl], start=True, stop=True)

                    kkm = wkp.tile([128, 2, C], F32, tag="kkm")
                    nc.vector.tensor_tensor(
                        kkm, pkk, mst_sb[:, None, :].to_broadcast((128, 2, C)),
                        ALU.mult)
                    nc.vector.tensor_tensor(kkm, kkm, e_l, ALU.mult)
                    a_low = wkp.tile([128, 2, C], BF16, tag="a_low")
                    nc.vector.tensor_tensor(
                        a_low, kkm,
                        beta_pair[:, :, None].to_broadcast((128, 2, C)), ALU.mult)
                    qkm = wkp.tile([128, 2, C], F32, tag="qkm")
                    nc.vector.tensor_tensor(
                        qkm, pqk, mui_sb[:, None, :].to_broadcast((128, 2, C)),
                        ALU.mult)
                    wqkT = wkp.tile([128, 2, C], BF16, tag="wqkT")
                    nc.vector.tensor_tensor(wqkT, qkm, e_t, ALU.mult)

                    pat = ps1.tile([128, 2, C], BF16, tag="aux")
                    for h in range(2):
                        nc.tensor.transpose(pat[:, h, :], a_low[:, h, :], id_bf)
                    a_up = wkp.tile([128, 2, C], BF16, tag="a_up")
                    nc.vector.tensor_copy(out=a_up, in_=pat)

                    lcur, ucur = a_low, a_up
                    u_chain = []
                    # 4 levels (powers to A^16) suffice: with l2-normalized k
                    # and sigmoid beta, ||A^16|| is negligible for all decay
                    # regimes (verified numerically incl. near-zero decay).
                    for lev in range(4):
                        last = lev == 3
                        plu = ps1.tile([128, 4, C], F32, tag="aux")
                        for h in range(2):
                            nc.tensor.matmul(plu[:, 2 + h, :], lhsT=lcur[:, h, :],
                                             rhs=ucur[:, h, :], start=True,
                                             stop=True)
                            if not last:
                                nc.tensor.matmul(plu[:, h, :], lhsT=ucur[:, h, :],
                                                 rhs=lcur[:, h, :], start=True,
                                                 stop=True)
                        lu = lup.tile([128, 4, C], BF16, tag=f"lu{lev}",
                                      name=f"lu{lev}")
                        if last:
                            nc.vector.tensor_copy(out=lu[:, 2:4, :],
                                                  in_=plu[:, 2:4, :])
                        else:
                            nc.vector.tensor_copy(out=lu, in_=plu)
                        lcur = lu[:, 0:2, :]
                        ucur = lu[:, 2:4, :]
                        u_chain.append(ucur)

                    pw1 = ps.tile([128, 2, VH], F32, tag="mm")
                    for h in range(2):
                        nc.tensor.matmul(pw1[:, h, :], lhsT=kn[h][:, csl],
                                         rhs=S0_bf[:, h, :], start=True, stop=True)
                    ytmp = wkp.tile([128, 2, VH], F32, tag="ytmp")
                    nc.vector.tensor_tensor(
                        ytmp, pw1, b_pair[:, :, None].to_broadcast((128, 2, VH)),
                        ALU.mult)
                    nc.vector.tensor_tensor(
                        ytmp, v_sb[:, ci, :].rearrange("p (h v) -> p h v", h=2),
                        ytmp, ALU.subtract)
                    y = wkp.tile([128, 2, VH], BF16, tag="y")
                    nc.vector.tensor_tensor(
                        y, ytmp, beta_pair[:, :, None].to_broadcast((128, 2, VH)),
                        ALU.mult)

                    for uj in reversed(u_chain):
                        pap = ps.tile([128, 2, VH], F32, tag="mm")
                        for h in range(2):
                            nc.tensor.matmul(pap[:, h, :], lhsT=uj[:, h, :],
                                             rhs=y[:, h, :], start=True, stop=True)
                        nc.vector.tensor_tensor(y, y, pap, ALU.add)
                    pap = ps.tile([128, 2, VH], F32, tag="mm")
                    for h in range(2):
                        nc.tensor.matmul(pap[:, h, :], lhsT=a_up[:, h, :],
                                         rhs=y[:, h, :], start=True, stop=True)
                    u_bf = wkp.tile([128, 2, VH], BF16, tag="u_bf")
                    nc.vector.tensor_tensor(u_bf, y, pap, ALU.subtract)

                    qt = wkp.tile([96, 2, C], BF16, tag="qt")
                    for h in range(2):
                        bbc = wkp.tile([96, C], F32, tag="bbc")
                        nc.gpsimd.partition_broadcast(
                            bbc, bT_rows[0:1, h, ci, :])
                        nc.vector.tensor_tensor(qt[:, h, :], qn[h][:, csl], bbc,
                                                ALU.mult)
                    po = pso.tile([96, 4, C], F32, tag="o")
                    for h in range(2):
                        for vh in range(2):
                            qi = 2 * h + vh
                            vs2 = slice(vh * 96, (vh + 1) * 96)
                            nc.tensor.matmul(po[:, qi, :], lhsT=S0_bf[:, h, vs2],
                                             rhs=qt[:, h, :], start=True,
                                             stop=False)
                            nc.tensor.matmul(po[:, qi, :], lhsT=u_bf[:, h, vs2],
                                             rhs=wqkT[:, h, :], start=False,
                                             stop=True)
                    nc.vector.tensor_copy(out=oT[:, :, csl], in_=po)

                    ur = wkp.tile([128, 2, VH], BF16, tag="ur")
                    nc.vector.tensor_tensor(
                        ur, u_bf, r_pair[:, :, None].to_broadcast((128, 2, VH)),
                        ALU.mult)
                    pstS = pso.tile([96, 2, VH], F32, tag="o")
                    for h in range(2):
                        pkc = ps1.tile([128, 96], BF16, tag="aux")
                        nc.tensor.transpose(pkc, kn[h][:, csl], id_bf[0:96, 0:96])
                        kc = wkp.tile([128, 96], BF16, tag="kc")
                        nc.vector.tensor_copy(out=kc, in_=pkc)
                        nc.tensor.matmul(pstS[:, h, :], lhsT=kc, rhs=ur[:, h, :],
                                         start=True, stop=True)
                        bcs = wkp.tile([96, 1], F32, tag="bcs")
                        nc.gpsimd.partition_broadcast(
                            bcs, bT_rows[0:1, h, ci, 127:128])
                        nc.vector.scalar_tensor_tensor(
                            out=S0[:, h, :], in0=S0[:, h, :], scalar=bcs,
                            in1=pstS[:, h, :], op0=ALU.mult, op1=ALU.add)
                    nc.vector.tensor_copy(out=S0_bf, in_=S0)

            # ================= phases C + D =================
            with (
                tc.tile_pool(name="cd1", bufs=1) as cd1,
                tc.tile_pool(name="cd2", bufs=2) as cd2,
            ):
                osq = cd1.tile([96, 4, T], BF16, tag="osq")
                nc.vector.tensor_tensor(osq, oT, oT, ALU.mult)
                ss_row = cd1.tile([1, T], F32, tag="ss_row")
                for tb in range(4):
                    tsl = slice(tb * TB, (tb + 1) * TB)
                    pss = ps1.tile([1, TB], F32, tag="aux")
                    for q in range(4):
                        nc.tensor.matmul(pss, lhsT=ones96, rhs=osq[:, q, tsl],
                                         start=(q == 0), stop=(q == 3))
                    nc.vector.tensor_copy(out=ss_row[:, tsl], in_=pss)
                nc.sync.dma_start(out=cc_in[:], in_=ss_row)
                if bool(__import__("os").environ.get("KB_NOCC")):
                    nc.sync.dma_start(out=cc_out[:], in_=cc_in[:])
                else:
                    nc.gpsimd.collective_compute(
                        "AllReduce", ALU.add, replica_groups=RG,
                        ins=[cc_in[:]], outs=[cc_out[:]])
                ssg = cd1.tile([1, T], F32, tag="ssg")
                nc.sync.dma_start(out=ssg, in_=cc_out[:])
                rms = cd1.tile([1, T], F32, tag="rms")
                nc.vector.tensor_scalar(out=rms, in0=ssg,
                                        scalar1=float(1.0 / 1536.0),
                                        scalar2=float(EPS), op0=ALU.mult,
                                        op1=ALU.add)
                nc.scalar.activation(rms, rms, AF.Sqrt)
                nc.vector.reciprocal(rms, rms)
                rms_bc = cd1.tile([96, T], F32, tag="rms_bc")
                nc.gpsimd.partition_broadcast(rms_bc, rms)
                if DBG:
                    nc.sync.dma_start(out=dbg_oT.rearrange("p (q t) -> p q t", q=4), in_=oT)
                    ds_ = dbg_ss.rearrange("p (k t) -> p k t", k=3)
                    nc.sync.dma_start(out=ds_[:, 0], in_=ss_row)
                    nc.sync.dma_start(out=ds_[:, 1], in_=ssg)
                    nc.sync.dma_start(out=ds_[:, 2], in_=rms)
                for q in range(4):
                    t1 = cd2.tile([96, T], BF16, tag="t1og")
                    nc.vector.scalar_tensor_tensor(
                        out=t1, in0=oT[:, q, :], scalar=pw_sb[:, q:q + 1],
                        in1=rms_bc, op0=ALU.mult, op1=ALU.mult)
                    nc.vector.tensor_tensor(gate_sb[:, q, :], gate_sb[:, q, :],
                                            t1, ALU.mult)

                for tt in range(NCH):
                    tsl = slice(tt * 128, (tt + 1) * 128)
                    for di in range(2):
                        dsl = slice(di * 512, (di + 1) * 512)
                        pout = pso.tile([128, 512], F32, tag="o")
                        for q in range(4):
                            nc.tensor.matmul(pout, lhsT=gate_sb[:, q, tsl],
                                             rhs=wo_sb[:, q, dsl],
                                             start=(q == 0), stop=(q == 3))
                        out_st = cd2.tile([128, 512], F32, tag="out_st")
                        nc.vector.tensor_copy(out=out_st, in_=pout)
                        nc.sync.dma_start(out=out_d[tsl, dsl], in_=out_st)

    nc.finalize()
    return nc


def make_in_maps(inputs):
    import ml_dtypes
    bfc = lambda a: np.ascontiguousarray(np.asarray(a, np.float32)).astype(
        ml_dtypes.bfloat16)
    f32 = lambda a: np.ascontiguousarray(np.asarray(a, np.float32))
    x = np.asarray(inputs["x"], np.float32)
    c_kv = np.asarray(inputs["c_kv"], np.float32)
    masks = np.stack([
        np.tril(np.ones((C, C), np.float32), -1),
        np.triu(np.ones((C, C), np.float32)),
    ])
    maps = []
    for core in range(8):
        b = core // 4
        hp = core % 4
        h0, h1 = 2 * hp, 2 * hp + 1
        qsl = slice(h0 * KH, (h1 + 1) * KH)
        vsl = slice(h0 * VH, (h1 + 1) * VH)
        wv = f32(inputs["w_v"])[:, vsl]
        cv = f32(inputs["conv_v_w"])[vsl, 0, :]
        wvs = np.stack([wv * cv[None, :, s] for s in range(4)])
        wab = np.stack([f32(inputs["a_proj_w"])[:, h0],
                        f32(inputs["a_proj_w"])[:, h1],
                        f32(inputs["b_proj_w"])[:, h0],
                        f32(inputs["b_proj_w"])[:, h1]], 1)
        abc = np.zeros((2, 3), np.float32)
        abc[:, 0] = -(f32(inputs["dt_bias"])[[h0, h1]] +
                      f32(inputs["a_proj_b"])[[h0, h1]])
        abc[:, 1] = np.exp(f32(inputs["A_log"]))[[h0, h1]]
        abc[:, 2] = f32(inputs["b_proj_b"])[[h0, h1]]
        maps.append({
            "xT": bfc(x[b].T),
            "ckvT": bfc(c_kv[b].T),
            "wq": bfc(f32(inputs["w_q"])[:, qsl]),
            "wk": bfc(f32(inputs["w_k"])[:, qsl]),
            "wvs": bfc(wvs),
            "wg": bfc(f32(inputs["g_proj_w"])[:, vsl]),
            "wo": bfc(f32(inputs["w_o"])[vsl, :]),
            "wab": bfc(wab),
            "convq": f32(inputs["conv_q_w"])[qsl, 0, :],
            "convk": f32(inputs["conv_k_w"])[qsl, 0, :],
            "cbq": f32(inputs["conv_q_b"])[qsl],
            "cbk": f32(inputs["conv_k_b"])[qsl],
            "vbias": f32(inputs["conv_v_b"])[vsl],
            "abc": abc,
            "pw": f32(inputs["post_norm_w"])[vsl],
            "masks": masks,
        })
    return maps


def assemble_output(results):
    out = np.zeros((2, T, DX), np.float32)
    for core in range(8):
        out[core // 4] += results[core]["out"]
    return out


# ======================================================================
# Runtime entry: kernel(**inputs) -> full (2, 2048, 1024) float32 output.
# Builds + compiles the SPMD program once per process, then reuses it.
# ======================================================================
_CACHE = {}


def _get_program():
    if "nc" not in _CACHE:
        _CACHE["nc"] = build_program()
    return _CACHE["nc"]


def build_noop_program():
    """Same I/O surface; reads every input byte but does no compute.

    Used to calibrate dispatch overhead: axon/PJRT input transfer happens
    lazily on first use, so the no-op must consume all inputs for the
    (full - noop) wall delta to isolate on-device execution.
    """
    nc = bacc.Bacc()
    specs = [
        ("xT", [DX, T], BF16), ("ckvT", [DKV, T], BF16),
        ("wq", [DX, 192], BF16), ("wk", [DKV, 192], BF16),
        ("wvs", [4, DKV, 384], BF16), ("wg", [DX, 384], BF16),
        ("wo", [384, DX], BF16), ("wab", [DX, 4], BF16),
        ("convq", [192, 4], F32), ("convk", [192, 4], F32),
        ("cbq", [192], F32), ("cbk", [192], F32), ("vbias", [384], F32),
        ("abc", [2, 3], F32), ("pw", [384], F32), ("masks", [2, C, C], F32),
    ]
    handles = {n: nc.declare_dram_parameter(n, s, d, isOutput=False)
               for n, s, d in specs}
    out_d = nc.declare_dram_parameter("out", [T, DX], F32, isOutput=True)
    with TileContext(nc) as tc:
        with tc.tile_pool(name="np1", bufs=3) as pool:
            for n, s, d in specs:
                h = handles[n]
                flat = int(np.prod(s))
                elt = 2 if d == BF16 else 4
                # stream through SBUF in [128, <=16384-byte] tiles
                ap = h[:] if len(s) == 1 else h.rearrange(
                    " ".join(f"d{i}" for i in range(len(s))) + " -> ("
                    + " ".join(f"d{i}" for i in range(len(s))) + ")")
                off = 0
                while off < flat:
                    rows = min(128, max(1, flat - off))
                    cols = min(8192 // elt, max(1, (flat - off) // rows))
                    take = rows * cols
                    if take == 0:
                        rows, cols, take = 1, flat - off, flat - off
                    tl = pool.tile([128, 8192 // elt], d, tag="t", name=f"t_{n}_{off}")
                    nc.sync.dma_start(
                        out=tl[0:rows, 0:cols],
                        in_=ap[off:off + take].rearrange("(r c) -> r c", c=cols))
                    off += take
            z = pool.tile([128, 512], F32, tag="z")
            nc.vector.memset(z, 0.0)
            nc.sync.dma_start(out=out_d[0:128, 0:512], in_=z)
    nc.finalize()
    return nc


def kernel(**inputs):
    from concourse.bass_utils import run_bass_kernel_spmd
    nc = _get_program()
    in_maps = make_in_maps(inputs)
    res = run_bass_kernel_spmd(nc, in_maps, list(range(8)))
    out = assemble_output([res.results[i] for i in range(8)])
    return out.astype(np.float32)


def run_once(in_maps, noop=False):
    """For benchmarking: one SPMD dispatch with prebuilt in_maps."""
    from concourse.bass_utils import run_bass_kernel_spmd
    if noop:
        if "noop" not in _CACHE:
            _CACHE["noop"] = build_noop_program()
        nc = _CACHE["noop"]
    else:
        nc = _get_program()
    return run_bass_kernel_spmd(nc, in_maps, list(range(8)))
